# revision 7
# baseline (speedup 1.0000x reference)
"""Tacotron2-style decoder on 8 Trainium2 NeuronCores.

Strategy: data-parallel over batch (32 -> 4 per core), all weights replicated.
Each core runs the full 500-step sequential scan for its batch slice.
Per step the two LSTM gate GEMMs run weights-as-moving-operand, col-tiled
across the four PE column groups (one gate type i/f/g/o per group), with
K-chunk accumulation in PSUM.  bf16 operands, fp32 accumulation/cell state.
The decoder-LSTM weight tail that does not fit in SBUF is re-streamed from
HBM every step (weights are step-invariant, so the stream is pure prefetch).
"""
import os
import sys
import dataclasses

sys.path.insert(0, "/opt/trn_rl_repo")

import numpy as np
import ml_dtypes

import concourse.bass as bass
import concourse.mybir as mybir
from concourse.tile import TileContext
from concourse.vector_clock import ScopedClock
from concourse.bass_utils import run_bass_kernel_spmd

BF = ml_dtypes.bfloat16
AF = mybir.ActivationFunctionType

# ---------------------------------------------------------------- workarounds
def _patched_drain_and_barrier(self, tick_clock, wait_clock):
    # this container's walrus accepts one sync-wait per instruction; split the
    # exit drain's waits across several drains.
    drain_inst = self.nc.sync.drain()
    wait_clock.add_sem_waits(drain_inst.ins, ScopedClock({None: tick_clock.global_clock}))
    si = drain_inst.ins.sync_info
    if si is not None and si.on_wait and len(si.on_wait) > 1:
        extra = si.on_wait[1:]
        del si.on_wait[1:]
        for w in extra:
            d2 = self.nc.sync.drain()
            si2 = d2.ins.sync_info
            if si2 is None:
                d2.ins.sync_info = type(si)(on_wait=[w], on_update=[])
            else:
                si2.on_wait.append(w)
    self.nc.all_engine_barrier()
    assert self.sems is not None
    popped = self.nc._tile_sem_poison_stack.pop()
    assert popped is self._sem_poison
    self.nc.clear_and_free_semaphores(list(self.sems.allocated().values()))
    self.nc.all_engine_barrier()


TileContext._drain_and_barrier = _patched_drain_and_barrier


def _fix_multiwaits(nc):
    """Hoist extra sync-waits onto same-engine NoOps (walrus 1-wait limit)."""
    n_fixed = 0
    for f in nc.m.functions:
        for b in f.blocks:
            out = []
            changed = False
            for inst in b.instructions:
                si = inst.sync_info
                if si is not None and si.on_wait is not None and len(si.on_wait) > 1:
                    waits = list(si.on_wait)
                    for k, w in enumerate(waits[:-1]):
                        out.append(mybir.InstNoOp(
                            name=f"{inst.name}-mwfix{k}",
                            engine=inst.engine,
                            sync_info=mybir.SyncInfo(on_wait=[w], on_update=[]),
                            bass_nofuse=True,
                        ))
                    inst.sync_info = mybir.SyncInfo(
                        on_wait=[waits[-1]], on_update=list(si.on_update or []))
                    changed = True
                    n_fixed += 1
                out.append(inst)
            if changed:
                b.instructions = out
    return n_fixed


# ---------------------------------------------------------------- dimensions
NCORES = 8
B = 32          # global batch
B4 = 4          # batch per core
T = 400         # encoder length
TP = 512        # padded encoder length (transpose blocks)
TC = 4          # t chunks of 128
S = int(os.environ.get("ATH_STEPS", "500"))
D = 512         # encoder dim
DC = 4
H = 1024        # lstm hidden
HC = 8
A = 128         # attention dim
G = 4096        # gates
PRE = 256
NM = 81         # mel 80 + stop 1
KS = 31
JK = 62
PAD = 15        # conv halo

KC_ATT = 14     # x(2) ctx(4) h(8)
KC_DEC = 20     # ha(8) ctx(4) hd(8)
KC_PS = 12      # hd(8) ctx(4)
RES_DEC = int(os.environ.get("ATH_RES_DEC", "0"))   # resident dec K-chunks
WBUFS = int(os.environ.get("ATH_WBUFS", "3"))       # streamed weight slots


def _windows_ap(tile_ap, row, nwin, width):
    """Overlapping-window read AP: [nwin, width] sliding (stride 1) over one
    SBUF partition row starting at `row` (for conv im2col via DMA)."""
    base = tile_ap[row:row + 1, 0:width]
    return dataclasses.replace(base, ap=[base.ap[0], [1, nwin], [1, width]])


def _build():
    nc = bass.Bass()
    dt = mybir.dt
    I = lambda n, s, d=dt.bfloat16: nc.dram_tensor(n, s, d, kind="ExternalInput")

    watt_d = I("watt", [128, KC_ATT * G])
    wdec_d = I("wdec", [128, KC_DEC * G])
    wq_d = I("wq", [128, HC * A])
    wps_d = I("wps", [128, KC_PS * NM])
    acv_d = I("acv", [128, 128])
    vb_d = I("vb", [128, 1])
    wpre1_d = I("wpre1", [128, PRE])
    wpre2_d = I("wpre2", [128, 2 * PRE])
    wmemT_d = I("wmemT", [128, DC * A])
    xraw_d = I("xraw", [128, 500 * B4])
    memsb_d = I("memsb", [128, TC * B4 * D])
    memT_d = I("memT", [128, DC * B4 * T])
    maskw_d = I("maskw", [128, T], dt.float32)

    melstop_d = nc.dram_tensor("melstop", [S, B4, NM], dt.float32, kind="ExternalOutput")
    aligns_d = nc.dram_tensor("aligns", [S, B4, T], dt.bfloat16, kind="ExternalOutput")

    with TileContext(nc) as tc:
        with tc.tile_pool(name="persist", bufs=1) as pp, \
             tc.tile_pool(name="state", bufs=1) as stp, \
             tc.tile_pool(name="dram", bufs=1, space="DRAM") as dp:
            # ---- resident weights/constants
            watt = pp.tile([128, KC_ATT * G], dt.bfloat16)
            nc.sync.dma_start(watt[:], watt_d[:])
            if RES_DEC:
                wdec_r = pp.tile([128, RES_DEC * G], dt.bfloat16)
                nc.sync.dma_start(wdec_r[:], wdec_d[:, :RES_DEC * G])
            wq = pp.tile([128, HC * A], dt.bfloat16)
            nc.sync.dma_start(wq[:], wq_d[:])
            wps = pp.tile([128, KC_PS * NM], dt.bfloat16)
            nc.sync.dma_start(wps[:], wps_d[:])
            acv = pp.tile([128, 128], dt.bfloat16)
            nc.sync.dma_start(acv[:], acv_d[:])
            vb = pp.tile([128, 1], dt.bfloat16)
            nc.sync.dma_start(vb[:], vb_d[:])
            memsb = pp.tile([128, TC * B4 * D], dt.bfloat16)
            nc.sync.dma_start(memsb[:], memsb_d[:])
            maskw = pp.tile([128, T], dt.float32)
            nc.sync.dma_start(maskw[:], maskw_d[:])
            pmem = pp.tile([128, B4 * T], dt.bfloat16)
            xdram = dp.tile([128, 2, 500 * B4], dt.bfloat16)

            # ---- state (zeroed)
            c_a = stp.tile([B4, H], dt.float32)
            c_d = stp.tile([B4, H], dt.float32)
            h_a = stp.tile([16, H], dt.bfloat16)
            h_d = stp.tile([16, H], dt.bfloat16)
            h_aT = stp.tile([128, HC, 16], dt.bfloat16)
            h_dT = stp.tile([128, HC, 16], dt.bfloat16)
            ctxT4 = stp.tile([128, DC * B4], dt.bfloat16)
            aw_pad = stp.tile([128, T + 2 * PAD], dt.bfloat16)
            awc_pad = stp.tile([128, T + 2 * PAD], dt.bfloat16)
            aw_f32 = stp.tile([128, T], dt.float32)
            awc_f32 = stp.tile([128, T], dt.float32)
            ew_tmp = stp.tile([128, T], dt.float32)
            aw_bf = stp.tile([128, TP], dt.bfloat16)
            for t_ in (c_a, c_d, h_a, h_d, aw_pad, awc_pad, aw_f32, awc_f32, ew_tmp, aw_bf):
                nc.gpsimd.memset(t_[:], 0.0)
            nc.gpsimd.memset(h_aT[:, :, :], 0.0)
            nc.gpsimd.memset(h_dT[:, :, :], 0.0)
            nc.gpsimd.memset(ctxT4[:], 0.0)

            # ---- init phase: prenet + pmem
            with tc.tile_pool(name="init", bufs=1) as ip, \
                 tc.tile_pool(name="initps", bufs=2, space="PSUM") as ipp:
                wpre1 = ip.tile([128, PRE], dt.bfloat16)
                nc.sync.dma_start(wpre1[:], wpre1_d[:])
                wpre2 = ip.tile([128, 2 * PRE], dt.bfloat16)
                nc.sync.dma_start(wpre2[:], wpre2_d[:])
                wmemT = ip.tile([128, DC * A], dt.bfloat16)
                nc.sync.dma_start(wmemT[:], wmemT_d[:])
                xraw = ip.tile([128, 500 * B4], dt.bfloat16)
                nc.sync.dma_start(xraw[:], xraw_d[:])
                x1 = ip.tile([128, 2, 500 * B4], dt.bfloat16)
                x2 = ip.tile([128, 2, 500 * B4], dt.bfloat16)
                NS = 500
                for c in range(2):
                    for n in range(2000 // NS):
                        ps = ipp.tile([128, NS], dt.float32, tag="ips")
                        nc.tensor.matmul(ps[:], wpre1[0:80, c * 128:(c + 1) * 128],
                                         xraw[0:80, n * NS:(n + 1) * NS])
                        nc.scalar.activation(x1[:, c, n * NS:(n + 1) * NS], ps[:], AF.Relu)
                for mc in range(2):
                    for n in range(2000 // NS):
                        ps = ipp.tile([128, NS], dt.float32, tag="ips")
                        for kc in range(2):
                            nc.tensor.matmul(
                                ps[:], wpre2[:, kc * PRE + mc * 128: kc * PRE + (mc + 1) * 128],
                                x1[:, kc, n * NS:(n + 1) * NS],
                                start=(kc == 0), stop=(kc == 1))
                        nc.scalar.activation(x2[:, mc, n * NS:(n + 1) * NS], ps[:], AF.Relu)
                nc.sync.dma_start(xdram[:, :, :], x2[:, :, :])
                memT = ip.tile([128, DC * B4 * T], dt.bfloat16)
                nc.sync.dma_start(memT[:], memT_d[:])
                for b in range(B4):
                    ps = ipp.tile([128, T], dt.float32, tag="ipm")
                    for dc in range(DC):
                        nc.tensor.matmul(ps[:], wmemT[:, dc * A:(dc + 1) * A],
                                         memT[:, (dc * B4 + b) * T:(dc * B4 + b + 1) * T],
                                         start=(dc == 0), stop=(dc == DC - 1))
                    nc.scalar.activation(pmem[:, b * T:(b + 1) * T], ps[:], AF.Copy)

            # ---- main loop pools
            with tc.tile_pool(name="wstream", bufs=WBUFS) as wsp, \
                 tc.tile_pool(name="gatt", bufs=1, space="PSUM") as gap, \
                 tc.tile_pool(name="gdec", bufs=1, space="PSUM") as gdp, \
                 tc.tile_pool(name="loc", bufs=2, space="PSUM") as locp, \
                 tc.tile_pool(name="small", bufs=2, space="PSUM") as smallp, \
                 tc.tile_pool(name="scratch", bufs=1) as scr:

                def step_body(iv):
                    # ---------- attention-LSTM gates
                    xcat_x = scr.tile([128, 2, 4], dt.bfloat16, tag="xcatx")
                    nc.sync.dma_start(xcat_x[:, :, :], xdram[:, :, bass.ts(iv, 4)])
                    gp_a = gap.tile([128, 1024], dt.float32, tag="gpa")

                    def att_lhs(kc):
                        if kc < 2:
                            return xcat_x[:, kc, 0:4]
                        if kc < 6:
                            return ctxT4[:, (kc - 2) * 4:(kc - 1) * 4]
                        return h_aT[:, kc - 6, 0:4]
                    for kc in range(KC_ATT):
                        lhs = att_lhs(kc)
                        for ty in range(4):
                            for hh in range(2):
                                nc.tensor.matmul(
                                    gp_a[32 * ty:32 * ty + 4, hh * 512:(hh + 1) * 512],
                                    lhs,
                                    watt[:, kc * G + ty * 1024 + hh * 512: kc * G + ty * 1024 + (hh + 1) * 512],
                                    start=(kc == 0), stop=(kc == KC_ATT - 1),
                                    tile_position=(0, 32 * ty))
                    # ---------- attention-LSTM cell
                    cs = scr.tile([B4, 4 * H], dt.bfloat16, tag="cell")
                    fc = scr.tile([B4, H], dt.float32, tag="fc")
                    ig = scr.tile([B4, H], dt.float32, tag="ig")
                    nc.scalar.activation(cs[:, 0:H], gp_a[0:4, :], AF.Sigmoid)            # i
                    nc.scalar.activation(cs[:, H:2 * H], gp_a[32:36, :], AF.Sigmoid)      # f
                    nc.scalar.activation(cs[:, 2 * H:3 * H], gp_a[64:68, :], AF.Tanh)     # g
                    nc.scalar.activation(cs[:, 3 * H:4 * H], gp_a[96:100, :], AF.Sigmoid)  # o
                    nc.vector.tensor_mul(fc[:], cs[:, H:2 * H], c_a[:])
                    nc.vector.tensor_mul(ig[:], cs[:, 0:H], cs[:, 2 * H:3 * H])
                    nc.vector.tensor_add(c_a[:], fc[:], ig[:])
                    nc.scalar.activation(fc[:], c_a[:], AF.Tanh)
                    nc.vector.tensor_mul(h_a[0:4, :], cs[:, 3 * H:4 * H], fc[:])
                    nc.sync.dma_start(h_aT[:, :, :], h_a[:], transpose=True)

                    # ---------- query + location conv + energies
                    qp = smallp.tile([128, 512], dt.float32, tag="qe")
                    for hc in range(HC):
                        nc.tensor.matmul(qp[:, 0:4], wq[:, hc * A:(hc + 1) * A],
                                         h_aT[:, hc, 0:4],
                                         start=(hc == 0), stop=(hc == HC - 1))
                    q_sb = scr.tile([128, 4], dt.float32, tag="qsb")
                    nc.scalar.activation(q_sb[:], qp[:, 0:4], AF.Copy)
                    im2 = scr.tile([JK, T], dt.bfloat16, tag="im2")
                    s1b = scr.tile([128, B4 * T], dt.bfloat16, tag="s1")
                    for b in range(B4):
                        nc.sync.dma_start(im2[0:KS, :], _windows_ap(aw_pad, 32 * b, KS, T))
                        nc.sync.dma_start(im2[KS:JK, :], _windows_ap(awc_pad, 32 * b, KS, T))
                        lp = locp.tile([128, T], dt.float32, tag="locp")
                        nc.tensor.matmul(lp[:], acv[0:JK, :], im2[:, :])
                        nc.vector.tensor_add(s1b[:, b * T:(b + 1) * T], lp[:], pmem[:, b * T:(b + 1) * T])
                        nc.scalar.activation(s1b[:, b * T:(b + 1) * T], s1b[:, b * T:(b + 1) * T],
                                             AF.Tanh, bias=q_sb[:, b:b + 1])
                    ep = smallp.tile([128, 512], dt.float32, tag="qe")
                    for b in range(B4):
                        nc.tensor.matmul(ep[32 * b:32 * b + 1, 0:T], vb[:, 0:1],
                                         s1b[:, b * T:(b + 1) * T], tile_position=(0, 32 * b))
                        nc.scalar.activation(ew_tmp[32 * b:32 * b + 1, :],
                                             ep[32 * b:32 * b + 1, 0:T], AF.Exp)
                    nc.vector.tensor_mul(ew_tmp[:], ew_tmp[:], maskw[:])
                    ssum = scr.tile([128, 1], dt.float32, tag="ssum")
                    nc.vector.reduce_sum(ssum[:], ew_tmp[:], axis=mybir.AxisListType.X)
                    rec = scr.tile([128, 1], dt.float32, tag="rec")
                    nc.vector.reciprocal(rec[:], ssum[:])
                    nc.vector.tensor_scalar_mul(aw_f32[:], ew_tmp[:], rec[:, 0:1])
                    nc.vector.tensor_add(awc_f32[:], awc_f32[:], aw_f32[:])
                    nc.vector.tensor_copy(aw_pad[:, PAD:PAD + T], aw_f32[:])
                    nc.vector.tensor_copy(awc_pad[:, PAD:PAD + T], awc_f32[:])
                    for b in range(B4):
                        nc.sync.dma_start(aligns_d[bass.ds(iv, 1), b:b + 1, :],
                                          aw_pad[32 * b:32 * b + 1, PAD:PAD + T])
                    # ---------- context
                    nc.vector.tensor_copy(aw_bf[:, 0:T], aw_f32[:])
                    awT = scr.tile([128, TC, 128], dt.bfloat16, tag="awT")
                    nc.sync.dma_start(awT[:, :, :], aw_bf[:], transpose=True)
                    cxp = smallp.tile([128, 512], dt.float32, tag="qe")
                    for b in range(B4):
                        for tcc in range(TC):
                            nc.tensor.matmul(
                                cxp[32 * b:32 * b + 1, 0:D], awT[:, tcc, 32 * b:32 * b + 1],
                                memsb[:, (tcc * B4 + b) * D:(tcc * B4 + b + 1) * D],
                                start=(tcc == 0), stop=(tcc == TC - 1),
                                tile_position=(0, 32 * b))
                    ctx_sb = scr.tile([128, D], dt.bfloat16, tag="ctxsb")
                    nc.scalar.activation(ctx_sb[:], cxp[:, 0:D], AF.Copy)
                    ctxT = scr.tile([128, DC, 128], dt.bfloat16, tag="ctxT")
                    nc.sync.dma_start(ctxT[:, :, :], ctx_sb[:], transpose=True)
                    for dc in range(DC):
                        for b in range(B4):
                            nc.vector.tensor_copy(ctxT4[:, dc * 4 + b:dc * 4 + b + 1],
                                                  ctxT[:, dc, 32 * b:32 * b + 1])

                    # ---------- decoder-LSTM gates
                    gp_d = gdp.tile([128, 1024], dt.float32, tag="gpd")

                    def dec_rhs(kc):
                        if kc < RES_DEC:
                            return wdec_r[:, kc * G:(kc + 1) * G]
                        wt = wsp.tile([128, G], dt.bfloat16, tag="wst")
                        nc.sync.dma_start(wt[:], wdec_d[:, kc * G:(kc + 1) * G])
                        return wt

                    def dec_lhs(kc):
                        if kc < 8:
                            return h_aT[:, kc, 0:4]
                        if kc < 12:
                            return ctxT4[:, (kc - 8) * 4:(kc - 7) * 4]
                        return h_dT[:, kc - 12, 0:4]
                    for kc in range(KC_DEC):
                        rhs = dec_rhs(kc)
                        lhs = dec_lhs(kc)
                        for ty in range(4):
                            for hh in range(2):
                                nc.tensor.matmul(
                                    gp_d[32 * ty:32 * ty + 4, hh * 512:(hh + 1) * 512],
                                    lhs,
                                    rhs[:, ty * 1024 + hh * 512: ty * 1024 + (hh + 1) * 512],
                                    start=(kc == 0), stop=(kc == KC_DEC - 1),
                                    tile_position=(0, 32 * ty))
                    # ---------- decoder cell
                    nc.scalar.activation(cs[:, 0:H], gp_d[0:4, :], AF.Sigmoid)
                    nc.scalar.activation(cs[:, H:2 * H], gp_d[32:36, :], AF.Sigmoid)
                    nc.scalar.activation(cs[:, 2 * H:3 * H], gp_d[64:68, :], AF.Tanh)
                    nc.scalar.activation(cs[:, 3 * H:4 * H], gp_d[96:100, :], AF.Sigmoid)
                    nc.vector.tensor_mul(fc[:], cs[:, H:2 * H], c_d[:])
                    nc.vector.tensor_mul(ig[:], cs[:, 0:H], cs[:, 2 * H:3 * H])
                    nc.vector.tensor_add(c_d[:], fc[:], ig[:])
                    nc.scalar.activation(fc[:], c_d[:], AF.Tanh)
                    nc.vector.tensor_mul(h_d[0:4, :], cs[:, 3 * H:4 * H], fc[:])
                    nc.sync.dma_start(h_dT[:, :, :], h_d[:], transpose=True)
                    # ---------- mel/stop projection
                    mp = smallp.tile([128, 512], dt.float32, tag="qe")
                    for kc in range(KC_PS):
                        lhs = h_dT[:, kc, 0:4] if kc < 8 else ctxT4[:, (kc - 8) * 4:(kc - 7) * 4]
                        nc.tensor.matmul(mp[0:4, 0:NM], lhs, wps[:, kc * NM:(kc + 1) * NM],
                                         start=(kc == 0), stop=(kc == KC_PS - 1))
                    mel_sb = scr.tile([B4, NM], dt.float32, tag="melsb")
                    nc.scalar.activation(mel_sb[:], mp[0:4, 0:NM], AF.Copy)
                    nc.sync.dma_start(melstop_d[bass.ds(iv, 1)], mel_sb[:])

                if os.environ.get("ATH_UNROLL_PY", "0") == "1":
                    for t in range(S):
                        step_body(t)
                else:
                    with tc.For_i(0, S, 1) as iv:
                        step_body(iv)

    _fix_multiwaits(nc)
    return nc


# ---------------------------------------------------------------- host side
def _pack_weights(p):
    f32 = lambda x: np.asarray(x, np.float32)
    W_aih, W_ahh = f32(p["W_aih"]), f32(p["W_ahh"])
    W_dih, W_dhh = f32(p["W_dih"]), f32(p["W_dhh"])
    WT_att = np.concatenate([W_aih.T, W_ahh.T], 0)              # [1792, 4096]
    watt = WT_att.reshape(KC_ATT, 128, G).transpose(1, 0, 2).reshape(128, KC_ATT * G)
    WT_dec = np.concatenate([W_dih.T, W_dhh.T], 0)              # [2560, 4096]
    wdec = WT_dec.reshape(KC_DEC, 128, G).transpose(1, 0, 2).reshape(128, KC_DEC * G)
    W_q = f32(p["W_q"])
    wq = W_q.T.reshape(HC, 128, A).transpose(1, 0, 2).reshape(128, HC * A)
    Wps = np.concatenate([f32(p["W_proj"]), f32(p["W_stop"])], 0)  # [81, 1536]
    wps = Wps.T.reshape(KC_PS, 128, NM).transpose(1, 0, 2).reshape(128, KC_PS * NM)
    A2 = np.einsum("af,fjk->ajk", f32(p["W_loc"]), f32(p["W_conv"])).reshape(128, JK)
    acv = np.zeros((128, 128), np.float32)
    acv[0:JK, :] = A2.T
    vb = f32(p["v"]).reshape(128, 1)
    wpre1 = np.zeros((128, PRE), np.float32)
    wpre1[0:80, :] = f32(p["W_pre1"]).T
    wpre2 = f32(p["W_pre2"]).T.reshape(2, 128, PRE).transpose(1, 0, 2).reshape(128, 2 * PRE)
    wmemT = f32(p["W_mem"]).T.reshape(DC, 128, A).transpose(1, 0, 2).reshape(128, DC * A)
    for nm in ("b_aih", "b_ahh", "b_dih", "b_dhh", "b_proj", "b_stop"):
        assert np.abs(f32(p[nm])).max() == 0.0, f"nonzero bias {nm} unsupported"
    bf = lambda x: np.ascontiguousarray(x.astype(BF))
    return {k: bf(v) for k, v in dict(
        watt=watt, wdec=wdec, wq=wq, wps=wps, acv=acv, vb=vb,
        wpre1=wpre1, wpre2=wpre2, wmemT=wmemT).items()}


def _pack_core(memory, dec_in, lens):
    """Per-core data tensors; memory [4,400,512] f32, dec_in [4,80,500]."""
    out = {}
    xraw = np.zeros((128, 500 * B4), np.float32)
    x = dec_in.transpose(2, 0, 1)                                # [500, 4, 80]
    xraw[0:80, B4:] = x[:-1].transpose(2, 0, 1).reshape(80, -1)  # shift: t>=1
    out["xraw"] = np.ascontiguousarray(xraw.astype(BF))
    memsb = np.zeros((128, TC * B4 * D), np.float32)
    for tcc in range(TC):
        n = min(128, T - tcc * 128)
        blk = memory[:, tcc * 128: tcc * 128 + n, :]             # [4, n, 512]
        memsb[:n, tcc * B4 * D:(tcc + 1) * B4 * D] = (
            blk.transpose(1, 0, 2).reshape(n, B4 * D))
    out["memsb"] = np.ascontiguousarray(memsb.astype(BF))
    memT = np.zeros((128, DC * B4 * T), np.float32)
    for dc in range(DC):
        blk = memory[:, :, dc * 128:(dc + 1) * 128]              # [4, 400, 128]
        memT[:, dc * B4 * T:(dc + 1) * B4 * T] = (
            blk.transpose(2, 0, 1).reshape(128, B4 * T))
    out["memT"] = np.ascontiguousarray(memT.astype(BF))
    maskw = np.zeros((128, T), np.float32)
    for b in range(B4):
        maskw[32 * b, :] = (np.arange(T) < int(lens[b])).astype(np.float32)
    out["maskw"] = maskw
    return out


_CACHED = {}


def kernel(memory, decoder_inputs, params, input_lengths):
    memory = np.asarray(memory, np.float32)
    decoder_inputs = np.asarray(decoder_inputs, np.float32)
    lens_in = np.asarray(input_lengths)
    wpack = _pack_weights(params)
    in_maps = []
    for c in range(NCORES):
        sl = slice(c * B4, (c + 1) * B4)
        m = dict(wpack)
        m.update(_pack_core(memory[sl], decoder_inputs[sl], lens_in[sl]))
        in_maps.append(m)
    if "prog" not in _CACHED:
        _CACHED["prog"] = _build()
    nc = _CACHED["prog"]
    res = run_bass_kernel_spmd(nc, in_maps, core_ids=list(range(NCORES)))
    mels = np.zeros((B, 80, S), np.float32)
    stops = np.zeros((B, S), np.float32)
    aligns = np.zeros((B, S, T), np.float32)
    for c in range(NCORES):
        r = res.results[c]
        ms = r["melstop"]                                        # [S, 4, 81]
        mels[c * B4:(c + 1) * B4] = ms[:, :, :80].transpose(1, 2, 0)
        stops[c * B4:(c + 1) * B4] = ms[:, :, 80].T
        aligns[c * B4:(c + 1) * B4] = np.asarray(r["aligns"], np.float32).transpose(1, 0, 2)
    return mels, stops, aligns
